# revision 2
# baseline (speedup 1.0000x reference)
"""GQA sliding-window attention (soft-cap + RoPE) on 8 Trainium2 NeuronCores.

Sharding: tensor-parallel over heads. Core c owns Q heads {2c, 2c+1} and KV
head c (GQA group stays local). Every core reads the full x (transposed and
cast to bf16 on the host), computes its two heads' attention and its slice of
the output projection, and writes a partial [T, D] fp32 output. The host sums
the 8 partials (the all-reduce implied by head-sharded o_w).

Device layout notes (all matmuls contract over the partition dim):
  - Projections produce q^T/k^T/v^T [head_dim, T] directly (weights are the
    stationary operand, x^T the moving one).
  - RoPE = q^T*cos + rot64(q^T)*sin_signed, where rot64 is a constant
    128x128 permutation matmul on the PE.
  - v^T is transposed back to natural [s, head_dim] tiles on the PE
    (needed as the stationary operand of the PV matmul).
  - Logits are computed transposed (l^T [s, t]) so the PV matmul consumes
    the probabilities without any transpose. Softmax denominators come from
    an accumulating ones-matmul; the reciprocal is broadcast across
    partitions with a rank-1 matmul.
"""

import sys

try:
    import concourse.bass as bass  # noqa: F401
except ImportError:  # fall back for environments without the axon PYTHONPATH
    sys.path.insert(0, "/opt/trn_rl_repo")

import ml_dtypes
import numpy as np

import concourse.bass as bass
import concourse.mybir as mybir
import concourse.tile as tile
from concourse import bacc
from concourse.bass_utils import run_bass_kernel_spmd

BF16 = ml_dtypes.bfloat16

B, T, D = 1, 4096, 2048
NUM_HEADS, NUM_KV_HEADS, HEAD_DIM = 16, 8, 128
N_CORES = 8
H_PER_CORE = NUM_HEADS // N_CORES  # 2
QUERY_PRE_ATTN_SCALAR = HEAD_DIM**-0.5
SOFT_CAP = 50.0
WINDOW = 1024
ROPE_BASE = 10000.0

TB = 512  # t-block (query block) size
NB = T // TB  # 8
DT = D // 128  # 16 d-tiles
NKT = T // 128  # 32 key tiles
KT_PER_B = TB // 128 + WINDOW // 128  # 12 key tiles cover one t-block's window


def _rope_tables(positions):
    """cos/sin tables in the transposed [head_dim, T] layout.

    row j (0..127) uses timescale index j%64; sin rows 0..63 carry -sin
    (they produce first-half outputs), rows 64..127 carry +sin.
    """
    j = np.arange(HEAD_DIM // 2, dtype=np.float64)
    timescale = ROPE_BASE ** (2.0 * j / HEAD_DIM)  # [64]
    ang = positions[None, :].astype(np.float64) / timescale[:, None]  # [64, T]
    cos = np.cos(ang)
    sin = np.sin(ang)
    cos2 = np.concatenate([cos, cos], axis=0)  # [128, T]
    sin_s = np.concatenate([-sin, sin], axis=0)  # [128, T]
    return cos2, sin_s


def _mask_tiles():
    """8 constant [128, 512] {0,1} tiles masking the window/causal edges.

    Pattern m corresponds to key-tile offset delta = kt - 4*b with
    delta in (-8,-7,-6,-5) for m in 0..3 (window edge) and delta in
    (0,1,2,3) for m in 4..7 (causal edge). allowed iff 0 <= t-s <= WINDOW-1
    with t-s = ft - 128*delta - ps.
    """
    deltas = [-8, -7, -6, -5, 0, 1, 2, 3]
    ps = np.arange(128)[:, None]
    ft = np.arange(TB)[None, :]
    tiles = []
    for d in deltas:
        diff = ft - 128 * d - ps
        allowed = (diff >= 0) & (diff <= WINDOW - 1)
        tiles.append(allowed.astype(np.float32))
    return np.stack(tiles, axis=1)  # [128, 8, 512]


def _build_program():
    nc = bacc.Bacc()
    dt = mybir.dt

    xt = nc.dram_tensor("xt", [128, DT, T], dt.bfloat16, kind="ExternalInput")
    wall = nc.dram_tensor("wall", [128, DT, 512], dt.bfloat16, kind="ExternalInput")
    ow = nc.dram_tensor("ow", [128, H_PER_CORE, D], dt.bfloat16, kind="ExternalInput")
    cosq = nc.dram_tensor("cosq", [128, T], dt.bfloat16, kind="ExternalInput")
    sinq = nc.dram_tensor("sinq", [128, T], dt.bfloat16, kind="ExternalInput")
    cosk = nc.dram_tensor("cosk", [128, T], dt.bfloat16, kind="ExternalInput")
    sink = nc.dram_tensor("sink", [128, T], dt.bfloat16, kind="ExternalInput")
    out = nc.dram_tensor("out", [T, D], dt.float32, kind="ExternalOutput")

    # constants identical on every core -> embed in the NEFF
    masks_np = _mask_tiles().astype(BF16)
    p64_np = np.zeros((128, 128), dtype=BF16)
    for m in range(128):
        p64_np[(m + 64) % 128, m] = 1.0
    ident_np = np.eye(128, dtype=BF16)
    masks_c = nc.inline_tensor(masks_np, name="masks")
    p64_c = nc.inline_tensor(p64_np, name="p64")
    ident_c = nc.inline_tensor(ident_np, name="ident")

    with tile.TileContext(nc) as tc:
        with (
            tc.tile_pool(name="consts", bufs=1) as consts,
            tc.tile_pool(name="persist", bufs=1) as persist,
            tc.tile_pool(name="xtp", bufs=2) as xtp,
            tc.tile_pool(name="stage", bufs=3) as stage,
            tc.tile_pool(name="ptile", bufs=2) as ptile,
            tc.tile_pool(name="outst", bufs=2) as outstp,
            tc.tile_pool(name="small", bufs=3) as small,
            tc.tile_pool(name="mm512", bufs=4, space="PSUM") as mmp,
            tc.tile_pool(name="encp", bufs=2, space="PSUM") as encp,
            tc.tile_pool(name="dnp", bufs=2, space="PSUM") as dnp,
        ):
            # ---- constants into SBUF ----
            wall_sb = consts.tile([128, DT, 512], dt.bfloat16)
            nc.sync.dma_start(out=wall_sb[:], in_=wall[:])
            ow_sb = consts.tile([128, H_PER_CORE, D], dt.bfloat16)
            nc.sync.dma_start(out=ow_sb[:], in_=ow[:])
            rope_sb = {}
            for nm, t_ in (("cosq", cosq), ("sinq", sinq), ("cosk", cosk), ("sink", sink)):
                s = consts.tile([128, T], dt.bfloat16, tag=nm)
                nc.sync.dma_start(out=s[:], in_=t_[:])
                rope_sb[nm] = s
            masks_sb = consts.tile([128, 8, TB], dt.bfloat16)
            nc.sync.dma_start(out=masks_sb[:], in_=masks_c[:])
            p64_sb = consts.tile([128, 128], dt.bfloat16)
            nc.sync.dma_start(out=p64_sb[:], in_=p64_c[:])
            ident_sb = consts.tile([128, 128], dt.bfloat16)
            nc.sync.dma_start(out=ident_sb[:], in_=ident_c[:])
            ones_col = consts.tile([128, 1], dt.bfloat16)
            nc.vector.memset(ones_col[:], 1.0)
            ones_row = consts.tile([1, 128], dt.float32)
            nc.vector.memset(ones_row[:], 1.0)

            # ---- persistent activations ----
            qt = [
                persist.tile([128, T], dt.bfloat16, tag=f"qt{h}", name=f"qt{h}")
                for h in range(2)
            ]
            kt_sb = persist.tile([128, T], dt.bfloat16, tag="kt")
            vn = persist.tile([128, T], dt.bfloat16, tag="vn")
            enc = [
                persist.tile([128, T], dt.bfloat16, tag=f"enc{h}", name=f"enc{h}")
                for h in range(2)
            ]

            # ---- phase B: projections + rope + v transpose ----
            rope_cfg = [  # (col index in wall, dest, cos, sin)
                (0, qt[0], "cosq", "sinq"),
                (1, qt[1], "cosq", "sinq"),
                (2, kt_sb, "cosk", "sink"),
            ]
            for b in range(NB):
                ts_ = slice(b * TB, (b + 1) * TB)
                xt_t = xtp.tile([128, DT, TB], dt.bfloat16, tag="xt")
                nc.sync.dma_start(out=xt_t[:], in_=xt[:, :, ts_])
                for ci in range(4):
                    pp = mmp.tile([128, TB], dt.float32, tag="mm")
                    for d_ in range(DT):
                        nc.tensor.matmul(
                            pp[:],
                            lhsT=wall_sb[:, d_, ci * 128 : (ci + 1) * 128],
                            rhs=xt_t[:, d_, :],
                            start=(d_ == 0),
                            stop=(d_ == DT - 1),
                        )
                    raw = stage.tile([128, TB], dt.bfloat16, tag="raw")
                    nc.scalar.copy(out=raw[:], in_=pp[:])
                    if ci == 3:
                        # v: transpose [c, s] -> natural [s, c] tiles
                        for i in range(TB // 128):
                            tp = mmp.tile([128, 128], dt.bfloat16, tag="mm")
                            nc.tensor.transpose(
                                tp[:], raw[:, i * 128 : (i + 1) * 128], ident_sb[:]
                            )
                            nc.vector.tensor_copy(
                                vn[:, b * TB + i * 128 : b * TB + (i + 1) * 128], tp[:]
                            )
                    else:
                        col, dest, cn, sn = ci, rope_cfg[ci][1], rope_cfg[ci][2], rope_cfg[ci][3]
                        rp = mmp.tile([128, TB], dt.float32, tag="mm")
                        nc.tensor.matmul(rp[:], lhsT=p64_sb[:], rhs=raw[:])
                        a_ = stage.tile([128, TB], dt.bfloat16, tag="ropea")
                        nc.vector.tensor_mul(a_[:], raw[:], rope_sb[cn][:, ts_])
                        b_ = stage.tile([128, TB], dt.bfloat16, tag="ropeb")
                        nc.vector.tensor_mul(b_[:], rp[:], rope_sb[sn][:, ts_])
                        nc.vector.tensor_add(dest[:, ts_], a_[:], b_[:])

            # ---- phase C: attention ----
            for b in range(NB):
                ts_ = slice(b * TB, (b + 1) * TB)
                kts = list(range(max(0, 4 * b - 8), 4 * b + 4))
                n = len(kts)
                for h in range(2):
                    p_t = ptile.tile([128, KT_PER_B * TB], dt.bfloat16, tag="p")
                    for i, kt_ in enumerate(kts):
                        lp = mmp.tile([128, TB], dt.float32, tag="mm")
                        nc.tensor.matmul(
                            lp[:],
                            lhsT=kt_sb[:, kt_ * 128 : (kt_ + 1) * 128],
                            rhs=qt[h][:, ts_],
                        )
                        nc.scalar.activation(
                            out=p_t[:, i * TB : (i + 1) * TB],
                            in_=lp[:],
                            func=mybir.ActivationFunctionType.Tanh,
                            scale=1.0 / SOFT_CAP,
                        )
                    pv = p_t[:, : n * TB]
                    nc.scalar.activation(
                        out=pv, in_=pv,
                        func=mybir.ActivationFunctionType.Exp,
                        scale=SOFT_CAP,
                    )
                    for i, kt_ in enumerate(kts):
                        delta = kt_ - 4 * b
                        if delta <= -5:
                            mi = delta + 8
                        elif delta >= 0:
                            mi = 4 + delta
                        else:
                            continue
                        sl = p_t[:, i * TB : (i + 1) * TB]
                        nc.vector.tensor_mul(sl, sl, masks_sb[:, mi, :])
                    ep = encp.tile([128, TB], dt.float32, tag="enc")
                    for i, kt_ in enumerate(kts):
                        nc.tensor.matmul(
                            ep[:],
                            lhsT=vn[:, kt_ * 128 : (kt_ + 1) * 128],
                            rhs=p_t[:, i * TB : (i + 1) * TB],
                            start=(i == 0),
                            stop=(i == n - 1),
                        )
                    dp = dnp.tile([1, TB], dt.float32, tag="dn")
                    for i in range(n):
                        nc.tensor.matmul(
                            dp[:],
                            lhsT=ones_col[:],
                            rhs=p_t[:, i * TB : (i + 1) * TB],
                            start=(i == 0),
                            stop=(i == n - 1),
                        )
                    rcp = small.tile([1, TB], dt.float32, tag="rcp")
                    nc.vector.reciprocal(rcp[:], dp[:])
                    rb = mmp.tile([128, TB], dt.float32, tag="mm")
                    nc.tensor.matmul(rb[:], lhsT=ones_row[:], rhs=rcp[:])
                    rbs = small.tile([128, TB], dt.float32, tag="rbs")
                    nc.vector.tensor_copy(rbs[:], rb[:])
                    nc.vector.tensor_mul(enc[h][:, ts_], ep[:], rbs[:])

            # ---- phase D: output projection (2 heads accumulated) ----
            for tt in range(NKT):
                o_sb = outstp.tile([128, D], dt.float32, tag="o")
                for dc in range(D // 512):
                    op = mmp.tile([128, 512], dt.float32, tag="mm")
                    for h in range(2):
                        nc.tensor.matmul(
                            op[:],
                            lhsT=enc[h][:, tt * 128 : (tt + 1) * 128],
                            rhs=ow_sb[:, h, dc * 512 : (dc + 1) * 512],
                            start=(h == 0),
                            stop=(h == 1),
                        )
                    nc.scalar.copy(
                        out=o_sb[:, dc * 512 : (dc + 1) * 512], in_=op[:]
                    )
                nc.sync.dma_start(
                    out=out[tt * 128 : (tt + 1) * 128, :], in_=o_sb[:]
                )

    nc.compile()
    return nc


_NC_CACHE = None


def _get_program():
    global _NC_CACHE
    if _NC_CACHE is None:
        _NC_CACHE = _build_program()
    return _NC_CACHE


def make_in_maps(x, segment_pos, q_w, kv_w, o_w):
    """Host-side shard prep: per-core input dicts."""
    positions = np.asarray(segment_pos)[0].astype(np.float64)
    cos2, sin_s = _rope_tables(positions)
    s = QUERY_PRE_ATTN_SCALAR
    cosq = (cos2 * s).astype(BF16)
    sinq = (sin_s * s).astype(BF16)
    cosk = cos2.astype(BF16)
    sink = sin_s.astype(BF16)

    xt = (
        np.asarray(x)[0].T.astype(BF16).reshape(DT, 128, T).transpose(1, 0, 2)
    )  # [128, DT, T], element [p, d_, t] = x[t, d_*128+p]
    xt = np.ascontiguousarray(xt)

    in_maps = []
    for c in range(N_CORES):
        w_cols = np.concatenate(
            [
                np.asarray(q_w)[2 * c],  # [D, 128]
                np.asarray(q_w)[2 * c + 1],
                np.asarray(kv_w)[0, c],
                np.asarray(kv_w)[1, c],
            ],
            axis=1,
        )  # [D, 512]
        wall = np.ascontiguousarray(
            w_cols.astype(BF16).reshape(DT, 128, 512).transpose(1, 0, 2)
        )  # [128, DT, 512], [p, d_, col] = W[d_*128+p, col]
        ow = np.ascontiguousarray(
            np.stack(
                [np.asarray(o_w)[2 * c], np.asarray(o_w)[2 * c + 1]], axis=1
            ).astype(BF16)
        )  # [128, 2, D]
        in_maps.append(
            {
                "xt": xt,
                "wall": wall,
                "ow": ow,
                "cosq": cosq,
                "sinq": sinq,
                "cosk": cosk,
                "sink": sink,
            }
        )
    return in_maps


def kernel(x, segment_pos, attn_mask, q_w, kv_w, o_w, _collect=None):
    nc = _get_program()
    in_maps = make_in_maps(x, segment_pos, q_w, kv_w, o_w)
    res = run_bass_kernel_spmd(nc, in_maps, core_ids=list(range(N_CORES)))
    if _collect is not None:
        _collect.append(res)
    acc = np.zeros((T, D), dtype=np.float32)
    for r in res.results:
        acc += r["out"]
    return acc.reshape(B, T, D)


# revision 8
# speedup vs baseline: 1.2455x; 1.2455x over previous
"""GQA sliding-window attention (soft-cap + RoPE) on 8 Trainium2 NeuronCores.

Sharding: tensor-parallel over heads. Core c owns Q heads {2c, 2c+1} and KV
head c (GQA group stays local). Every core reads the full x (transposed and
cast to bf16 on the host), computes its two heads' attention and its slice of
the output projection, and writes a partial [T, D] fp32 output. The host sums
the 8 partials (the all-reduce implied by head-sharded o_w).

Device layout notes (all matmuls contract over the partition dim):
  - Projections produce q^T/k^T/v^T [head_dim, T] directly (weights are the
    stationary operand, x^T the moving one).
  - RoPE = q^T*cos + rot64(q^T)*sin_signed, where rot64 is a constant
    128x128 permutation matmul on the PE.
  - v^T is transposed back to natural [s, head_dim] tiles on the PE
    (needed as the stationary operand of the PV matmul).
  - Logits are computed transposed (l^T [s, t]) so the PV matmul consumes
    the probabilities without any transpose. Softmax denominators come from
    an accumulating ones-matmul; the reciprocal is broadcast across
    partitions with a rank-1 matmul.
"""

import sys

try:
    import concourse.bass as bass  # noqa: F401
except ImportError:  # fall back for environments without the axon PYTHONPATH
    sys.path.insert(0, "/opt/trn_rl_repo")

import ml_dtypes
import numpy as np

import concourse.bass as bass
import concourse.mybir as mybir
import concourse.tile as tile
from concourse import bacc
from concourse.bass_utils import run_bass_kernel_spmd

BF16 = ml_dtypes.bfloat16

B, T, D = 1, 4096, 2048
NUM_HEADS, NUM_KV_HEADS, HEAD_DIM = 16, 8, 128
N_CORES = 8
H_PER_CORE = NUM_HEADS // N_CORES  # 2
QUERY_PRE_ATTN_SCALAR = HEAD_DIM**-0.5
SOFT_CAP = 50.0
WINDOW = 1024
ROPE_BASE = 10000.0

TB = 512  # t-block (query block) size
NB = T // TB  # 8
DT = D // 128  # 16 d-tiles
NKT = T // 128  # 32 key tiles
KT_PER_B = TB // 128 + WINDOW // 128  # 12 key tiles cover one t-block's window


def _rope_tables(positions):
    """cos/sin tables in the transposed [head_dim, T] layout.

    row j (0..127) uses timescale index j%64; sin rows 0..63 carry -sin
    (they produce first-half outputs), rows 64..127 carry +sin.
    """
    j = np.arange(HEAD_DIM // 2, dtype=np.float64)
    timescale = ROPE_BASE ** (2.0 * j / HEAD_DIM)  # [64]
    ang = positions[None, :].astype(np.float64) / timescale[:, None]  # [64, T]
    cos = np.cos(ang)
    sin = np.sin(ang)
    cos2 = np.concatenate([cos, cos], axis=0)  # [128, T]
    sin_s = np.concatenate([-sin, sin], axis=0)  # [128, T]
    return cos2, sin_s


def _mask_tiles():
    """8 constant [128, 512] {0,1} tiles masking the window/causal edges.

    Pattern m corresponds to key-tile offset delta = kt - 4*b with
    delta in (-8,-7,-6,-5) for m in 0..3 (window edge) and delta in
    (0,1,2,3) for m in 4..7 (causal edge). allowed iff 0 <= t-s <= WINDOW-1
    with t-s = ft - 128*delta - ps.
    """
    deltas = [-8, -7, -6, -5, 0, 1, 2, 3]
    ps = np.arange(128)[:, None]
    ft = np.arange(TB)[None, :]
    tiles = []
    for d in deltas:
        diff = ft - 128 * d - ps
        allowed = (diff >= 0) & (diff <= WINDOW - 1)
        tiles.append(allowed.astype(np.float32))
    return np.stack(tiles, axis=1)  # [128, 8, 512]


def _build_program():
    nc = bacc.Bacc()
    dt = mybir.dt

    xt = nc.dram_tensor("xt", [128, DT, T], dt.bfloat16, kind="ExternalInput")
    wall = nc.dram_tensor("wall", [128, DT, 512], dt.bfloat16, kind="ExternalInput")
    ow = nc.dram_tensor("ow", [128, H_PER_CORE, D], dt.bfloat16, kind="ExternalInput")
    cosq = nc.dram_tensor("cosq", [128, T], dt.bfloat16, kind="ExternalInput")
    sinq = nc.dram_tensor("sinq", [128, T], dt.bfloat16, kind="ExternalInput")
    cosk = nc.dram_tensor("cosk", [128, T], dt.bfloat16, kind="ExternalInput")
    sink = nc.dram_tensor("sink", [128, T], dt.bfloat16, kind="ExternalInput")
    out = nc.dram_tensor("out", [T, D], dt.float32, kind="ExternalOutput")

    # constants identical on every core -> embed in the NEFF
    masks_np = _mask_tiles().astype(BF16)
    p64_np = np.zeros((128, 128), dtype=BF16)
    for m in range(128):
        p64_np[(m + 64) % 128, m] = 1.0
    ident_np = np.eye(128, dtype=BF16)
    masks_c = nc.inline_tensor(masks_np, name="masks")
    p64_c = nc.inline_tensor(p64_np, name="p64")
    ident_c = nc.inline_tensor(ident_np, name="ident")

    with tile.TileContext(nc) as tc:
        with (
            tc.tile_pool(name="consts", bufs=1) as consts,
            tc.tile_pool(name="persist", bufs=1) as persist,
            tc.tile_pool(name="xtp", bufs=2) as xtp,
            tc.tile_pool(name="stage", bufs=3) as stage,
            tc.tile_pool(name="ptile", bufs=2) as ptile,
            tc.tile_pool(name="outst", bufs=2) as outstp,
            tc.tile_pool(name="small", bufs=3) as small,
            tc.tile_pool(name="mm512", bufs=2, space="PSUM") as mmp,
            tc.tile_pool(name="ltp", bufs=4, space="PSUM") as ltp,
            tc.tile_pool(name="encp", bufs=1, space="PSUM") as encp,
            tc.tile_pool(name="dnp", bufs=1, space="PSUM") as dnp,
        ):
            # ---- constants into SBUF ----
            wall_sb = consts.tile([128, DT, 512], dt.bfloat16)
            nc.sync.dma_start(out=wall_sb[:], in_=wall[:])
            ow_sb = consts.tile([128, H_PER_CORE, D], dt.bfloat16)
            nc.sync.dma_start(out=ow_sb[:], in_=ow[:])
            rope_sb = {}
            for nm, t_ in (("cosq", cosq), ("sinq", sinq), ("cosk", cosk), ("sink", sink)):
                s = consts.tile([128, T], dt.bfloat16, tag=nm)
                nc.sync.dma_start(out=s[:], in_=t_[:])
                rope_sb[nm] = s
            masks_sb = consts.tile([128, 8, TB], dt.bfloat16)
            nc.sync.dma_start(out=masks_sb[:], in_=masks_c[:])
            p64_sb = consts.tile([128, 128], dt.bfloat16)
            nc.sync.dma_start(out=p64_sb[:], in_=p64_c[:])
            ident_sb = consts.tile([128, 128], dt.bfloat16)
            nc.sync.dma_start(out=ident_sb[:], in_=ident_c[:])
            ones_col = consts.tile([128, 1], dt.bfloat16)
            nc.vector.memset(ones_col[:], 1.0)
            ones_row = consts.tile([1, 128], dt.float32)
            nc.vector.memset(ones_row[:], 1.0)

            # ---- persistent activations ----
            qt = [
                persist.tile([128, T], dt.bfloat16, tag=f"qt{h}", name=f"qt{h}")
                for h in range(2)
            ]
            kt_sb = persist.tile([128, T], dt.bfloat16, tag="kt")
            vn = persist.tile([128, T], dt.bfloat16, tag="vn")
            enc = [
                persist.tile([128, T], dt.bfloat16, tag=f"enc{h}", name=f"enc{h}")
                for h in range(2)
            ]

            # ---- phase B: projections + rope + v transpose ----
            rope_cfg = [  # (col index in wall, dest, cos, sin)
                (0, qt[0], "cosq", "sinq"),
                (1, qt[1], "cosq", "sinq"),
                (2, kt_sb, "cosk", "sink"),
            ]
            for b in range(NB):
                ts_ = slice(b * TB, (b + 1) * TB)
                xt_t = xtp.tile([128, DT, TB], dt.bfloat16, tag="xt")
                nc.sync.dma_start(out=xt_t[:], in_=xt[:, :, ts_])
                for ci in range(4):
                    pp = mmp.tile([128, TB], dt.float32, tag="mm")
                    for d_ in range(DT):
                        nc.tensor.matmul(
                            pp[:],
                            lhsT=wall_sb[:, d_, ci * 128 : (ci + 1) * 128],
                            rhs=xt_t[:, d_, :],
                            start=(d_ == 0),
                            stop=(d_ == DT - 1),
                        )
                    raw = stage.tile([128, TB], dt.bfloat16, tag="raw")
                    nc.vector.tensor_copy(raw[:], pp[:])
                    if ci == 3:
                        # v: transpose [c, s] -> natural [s, c] tiles
                        for i in range(TB // 128):
                            tp = mmp.tile([128, 128], dt.bfloat16, tag="mm")
                            nc.tensor.transpose(
                                tp[:], raw[:, i * 128 : (i + 1) * 128], ident_sb[:]
                            )
                            nc.vector.tensor_copy(
                                vn[:, b * TB + i * 128 : b * TB + (i + 1) * 128], tp[:]
                            )
                    else:
                        col, dest, cn, sn = ci, rope_cfg[ci][1], rope_cfg[ci][2], rope_cfg[ci][3]
                        rp = mmp.tile([128, TB], dt.float32, tag="mm")
                        nc.tensor.matmul(rp[:], lhsT=p64_sb[:], rhs=raw[:])
                        a_ = stage.tile([128, TB], dt.bfloat16, tag="ropea")
                        nc.vector.tensor_mul(a_[:], raw[:], rope_sb[cn][:, ts_])
                        b_ = stage.tile([128, TB], dt.bfloat16, tag="ropeb")
                        nc.vector.tensor_mul(b_[:], rp[:], rope_sb[sn][:, ts_])
                        nc.vector.tensor_add(dest[:, ts_], a_[:], b_[:])

            # ---- phase C: attention ----
            for b in range(NB):
                ts_ = slice(b * TB, (b + 1) * TB)
                kts = list(range(max(0, 4 * b - 8), 4 * b + 4))
                n = len(kts)
                for h in range(2):
                    p_t = ptile.tile([128, KT_PER_B * TB], dt.bfloat16, tag="p")
                    for i, kt_ in enumerate(kts):
                        lp = ltp.tile([128, TB], dt.float32, tag="lt")
                        nc.tensor.matmul(
                            lp[:],
                            lhsT=kt_sb[:, kt_ * 128 : (kt_ + 1) * 128],
                            rhs=qt[h][:, ts_],
                        )
                        nc.scalar.activation(
                            out=p_t[:, i * TB : (i + 1) * TB],
                            in_=lp[:],
                            func=mybir.ActivationFunctionType.Tanh,
                            scale=1.0 / SOFT_CAP,
                        )
                    pv = p_t[:, : n * TB]
                    nc.scalar.activation(
                        out=pv, in_=pv,
                        func=mybir.ActivationFunctionType.Exp,
                        scale=SOFT_CAP,
                    )
                    for i, kt_ in enumerate(kts):
                        delta = kt_ - 4 * b
                        if delta <= -5:
                            mi = delta + 8
                        elif delta >= 0:
                            mi = 4 + delta
                        else:
                            continue
                        sl = p_t[:, i * TB : (i + 1) * TB]
                        nc.vector.tensor_mul(sl, sl, masks_sb[:, mi, :])
                    ep = encp.tile([128, TB], dt.float32, tag="enc")
                    for i, kt_ in enumerate(kts):
                        nc.tensor.matmul(
                            ep[:],
                            lhsT=vn[:, kt_ * 128 : (kt_ + 1) * 128],
                            rhs=p_t[:, i * TB : (i + 1) * TB],
                            start=(i == 0),
                            stop=(i == n - 1),
                        )
                    dp = dnp.tile([1, TB], dt.float32, tag="dn")
                    for i in range(n):
                        nc.tensor.matmul(
                            dp[:],
                            lhsT=ones_col[:],
                            rhs=p_t[:, i * TB : (i + 1) * TB],
                            start=(i == 0),
                            stop=(i == n - 1),
                        )
                    rcp = small.tile([1, TB], dt.float32, tag="rcp", bufs=2)
                    rscr = small.tile([1, TB], dt.float32, tag="rscr", bufs=1)
                    nc.vector.reciprocal_approx_accurate(rcp[:], dp[:], rscr[:])
                    rb = mmp.tile([128, TB], dt.float32, tag="mm")
                    nc.tensor.matmul(rb[:], lhsT=ones_row[:], rhs=rcp[:])
                    rbs = small.tile([128, TB], dt.float32, tag="rbs", bufs=2)
                    nc.vector.tensor_copy(rbs[:], rb[:])
                    nc.vector.tensor_mul(enc[h][:, ts_], ep[:], rbs[:])

            # ---- phase D: output projection (2 heads accumulated) ----
            for tt in range(NKT):
                o_sb = outstp.tile([128, D], dt.float32, tag="o")
                for dc in range(D // 512):
                    op = mmp.tile([128, 512], dt.float32, tag="mm")
                    for h in range(2):
                        nc.tensor.matmul(
                            op[:],
                            lhsT=enc[h][:, tt * 128 : (tt + 1) * 128],
                            rhs=ow_sb[:, h, dc * 512 : (dc + 1) * 512],
                            start=(h == 0),
                            stop=(h == 1),
                        )
                    nc.scalar.copy(
                        out=o_sb[:, dc * 512 : (dc + 1) * 512], in_=op[:]
                    )
                nc.sync.dma_start(
                    out=out[tt * 128 : (tt + 1) * 128, :], in_=o_sb[:]
                )

    nc.compile()
    return nc


_NC_CACHE = None


def _get_program():
    global _NC_CACHE
    if _NC_CACHE is None:
        _NC_CACHE = _build_program()
    return _NC_CACHE


def make_in_maps(x, segment_pos, q_w, kv_w, o_w):
    """Host-side shard prep: per-core input dicts."""
    positions = np.asarray(segment_pos)[0].astype(np.float64)
    cos2, sin_s = _rope_tables(positions)
    s = QUERY_PRE_ATTN_SCALAR
    cosq = (cos2 * s).astype(BF16)
    sinq = (sin_s * s).astype(BF16)
    cosk = cos2.astype(BF16)
    sink = sin_s.astype(BF16)

    xt = (
        np.asarray(x)[0].T.astype(BF16).reshape(DT, 128, T).transpose(1, 0, 2)
    )  # [128, DT, T], element [p, d_, t] = x[t, d_*128+p]
    xt = np.ascontiguousarray(xt)

    in_maps = []
    for c in range(N_CORES):
        w_cols = np.concatenate(
            [
                np.asarray(q_w)[2 * c],  # [D, 128]
                np.asarray(q_w)[2 * c + 1],
                np.asarray(kv_w)[0, c],
                np.asarray(kv_w)[1, c],
            ],
            axis=1,
        )  # [D, 512]
        wall = np.ascontiguousarray(
            w_cols.astype(BF16).reshape(DT, 128, 512).transpose(1, 0, 2)
        )  # [128, DT, 512], [p, d_, col] = W[d_*128+p, col]
        ow = np.ascontiguousarray(
            np.stack(
                [np.asarray(o_w)[2 * c], np.asarray(o_w)[2 * c + 1]], axis=1
            ).astype(BF16)
        )  # [128, 2, D]
        in_maps.append(
            {
                "xt": xt,
                "wall": wall,
                "ow": ow,
                "cosq": cosq,
                "sinq": sinq,
                "cosk": cosk,
                "sink": sink,
            }
        )
    return in_maps


def kernel(x, segment_pos, attn_mask, q_w, kv_w, o_w, _collect=None):
    nc = _get_program()
    in_maps = make_in_maps(x, segment_pos, q_w, kv_w, o_w)
    res = run_bass_kernel_spmd(nc, in_maps, core_ids=list(range(N_CORES)))
    if _collect is not None:
        _collect.append(res)
    acc = np.zeros((T, D), dtype=np.float32)
    for r in res.results:
        acc += r["out"]
    return acc.reshape(B, T, D)


# revision 9
# speedup vs baseline: 1.4279x; 1.1465x over previous
"""GQA sliding-window attention (soft-cap + RoPE) on 8 Trainium2 NeuronCores.

Sharding: tensor-parallel over heads. Core c owns Q heads {2c, 2c+1} and KV
head c (GQA group stays local). Every core reads the full x (transposed and
cast to bf16 on the host), computes its two heads' attention and its slice of
the output projection, and writes a partial [T, D] fp32 output. The host sums
the 8 partials (the all-reduce implied by head-sharded o_w).

Device layout notes (all matmuls contract over the partition dim):
  - Projections produce q^T/k^T/v^T [head_dim, T] directly (weights are the
    stationary operand, x^T the moving one).
  - RoPE = q^T*cos + rot64(q^T)*sin_signed, where rot64 is a constant
    128x128 permutation matmul on the PE.
  - v^T is transposed back to natural [s, head_dim] tiles on the PE
    (needed as the stationary operand of the PV matmul).
  - Logits are computed transposed (l^T [s, t]) so the PV matmul consumes
    the probabilities without any transpose. Softmax denominators come from
    an accumulating ones-matmul; the reciprocal is broadcast across
    partitions with a rank-1 matmul.
"""

import sys

try:
    import concourse.bass as bass  # noqa: F401
except ImportError:  # fall back for environments without the axon PYTHONPATH
    sys.path.insert(0, "/opt/trn_rl_repo")

import ml_dtypes
import numpy as np

import concourse.bass as bass
import concourse.mybir as mybir
import concourse.tile as tile
from concourse import bacc
from concourse.bass_utils import run_bass_kernel_spmd

BF16 = ml_dtypes.bfloat16

B, T, D = 1, 4096, 2048
NUM_HEADS, NUM_KV_HEADS, HEAD_DIM = 16, 8, 128
N_CORES = 8
H_PER_CORE = NUM_HEADS // N_CORES  # 2
QUERY_PRE_ATTN_SCALAR = HEAD_DIM**-0.5
SOFT_CAP = 50.0
WINDOW = 1024
ROPE_BASE = 10000.0

TB = 512  # t-block (query block) size
NB = T // TB  # 8
DT = D // 128  # 16 d-tiles
NKT = T // 128  # 32 key tiles
KT_PER_B = TB // 128 + WINDOW // 128  # 12 key tiles cover one t-block's window


def _rope_tables(positions):
    """cos/sin tables in the transposed [head_dim, T] layout.

    row j (0..127) uses timescale index j%64; sin rows 0..63 carry -sin
    (they produce first-half outputs), rows 64..127 carry +sin.
    """
    j = np.arange(HEAD_DIM // 2, dtype=np.float64)
    timescale = ROPE_BASE ** (2.0 * j / HEAD_DIM)  # [64]
    ang = positions[None, :].astype(np.float64) / timescale[:, None]  # [64, T]
    cos = np.cos(ang)
    sin = np.sin(ang)
    cos2 = np.concatenate([cos, cos], axis=0)  # [128, T]
    sin_s = np.concatenate([-sin, sin], axis=0)  # [128, T]
    return cos2, sin_s


def _mask_tiles():
    """8 constant [128, 512] {0,1} tiles masking the window/causal edges.

    Pattern m corresponds to key-tile offset delta = kt - 4*b with
    delta in (-8,-7,-6,-5) for m in 0..3 (window edge) and delta in
    (0,1,2,3) for m in 4..7 (causal edge). allowed iff 0 <= t-s <= WINDOW-1
    with t-s = ft - 128*delta - ps.
    """
    deltas = [-8, -7, -6, -5, 0, 1, 2, 3]
    ps = np.arange(128)[:, None]
    ft = np.arange(TB)[None, :]
    tiles = []
    for d in deltas:
        diff = ft - 128 * d - ps
        allowed = (diff >= 0) & (diff <= WINDOW - 1)
        tiles.append(allowed.astype(np.float32))
    return np.stack(tiles, axis=1)  # [128, 8, 512]


def _build_program():
    nc = bacc.Bacc()
    dt = mybir.dt

    xt = nc.dram_tensor("xt", [128, DT, T], dt.bfloat16, kind="ExternalInput")
    wall = nc.dram_tensor("wall", [128, DT, 512], dt.bfloat16, kind="ExternalInput")
    ow = nc.dram_tensor("ow", [128, H_PER_CORE, D], dt.bfloat16, kind="ExternalInput")
    cosq = nc.dram_tensor("cosq", [128, T], dt.bfloat16, kind="ExternalInput")
    sinq = nc.dram_tensor("sinq", [128, T], dt.bfloat16, kind="ExternalInput")
    cosk = nc.dram_tensor("cosk", [128, T], dt.bfloat16, kind="ExternalInput")
    sink = nc.dram_tensor("sink", [128, T], dt.bfloat16, kind="ExternalInput")
    out = nc.dram_tensor("out", [T, D], dt.float32, kind="ExternalOutput")

    # constants identical on every core -> embed in the NEFF
    masks_np = _mask_tiles().astype(BF16)
    p64_np = np.zeros((128, 128), dtype=BF16)
    for m in range(128):
        p64_np[(m + 64) % 128, m] = 1.0
    ident_np = np.eye(128, dtype=BF16)
    masks_c = nc.inline_tensor(masks_np, name="masks")
    p64_c = nc.inline_tensor(p64_np, name="p64")
    ident_c = nc.inline_tensor(ident_np, name="ident")

    with tile.TileContext(nc) as tc:
        with (
            tc.tile_pool(name="consts", bufs=1) as consts,
            tc.tile_pool(name="persist", bufs=1) as persist,
            tc.tile_pool(name="xtp", bufs=2) as xtp,
            tc.tile_pool(name="stage", bufs=3) as stage,
            tc.tile_pool(name="ptile", bufs=2) as ptile,
            tc.tile_pool(name="outst", bufs=2) as outstp,
            tc.tile_pool(name="small", bufs=3) as small,
            tc.tile_pool(name="mm512", bufs=2, space="PSUM") as mmp,
            tc.tile_pool(name="ltp", bufs=4, space="PSUM") as ltp,
            tc.tile_pool(name="encp", bufs=1, space="PSUM") as encp,
            tc.tile_pool(name="dnp", bufs=1, space="PSUM") as dnp,
        ):
            # ---- constants into SBUF ----
            wall_sb = consts.tile([128, DT, 512], dt.bfloat16)
            nc.sync.dma_start(out=wall_sb[:], in_=wall[:])
            ow_sb = consts.tile([128, H_PER_CORE, D], dt.bfloat16)
            nc.sync.dma_start(out=ow_sb[:], in_=ow[:])
            rope_sb = {}
            for nm, t_ in (("cosq", cosq), ("sinq", sinq), ("cosk", cosk), ("sink", sink)):
                s = consts.tile([128, T], dt.bfloat16, tag=nm)
                nc.sync.dma_start(out=s[:], in_=t_[:])
                rope_sb[nm] = s
            masks_sb = consts.tile([128, 8, TB], dt.bfloat16)
            nc.sync.dma_start(out=masks_sb[:], in_=masks_c[:])
            p64_sb = consts.tile([128, 128], dt.bfloat16)
            nc.sync.dma_start(out=p64_sb[:], in_=p64_c[:])
            ident_sb = consts.tile([128, 128], dt.bfloat16)
            nc.sync.dma_start(out=ident_sb[:], in_=ident_c[:])
            ones_col = consts.tile([128, 1], dt.bfloat16)
            nc.vector.memset(ones_col[:], 1.0)
            ones_row = consts.tile([1, 128], dt.float32)
            nc.vector.memset(ones_row[:], 1.0)

            # ---- persistent activations ----
            qt = [
                persist.tile([128, T], dt.bfloat16, tag=f"qt{h}", name=f"qt{h}")
                for h in range(2)
            ]
            kt_sb = persist.tile([128, T], dt.bfloat16, tag="kt")
            vn = persist.tile([128, T], dt.bfloat16, tag="vn")
            enc = [
                persist.tile([128, T], dt.bfloat16, tag=f"enc{h}", name=f"enc{h}")
                for h in range(2)
            ]

            # ---- phase B: projections + rope + v transpose ----
            rope_cfg = [  # (col index in wall, dest, cos, sin)
                (0, qt[0], "cosq", "sinq"),
                (1, qt[1], "cosq", "sinq"),
                (2, kt_sb, "cosk", "sink"),
            ]
            for b in range(NB):
                ts_ = slice(b * TB, (b + 1) * TB)
                xt_t = xtp.tile([128, DT, TB], dt.bfloat16, tag="xt")
                nc.sync.dma_start(out=xt_t[:], in_=xt[:, :, ts_])
                for ci in range(4):
                    pp = mmp.tile([128, TB], dt.float32, tag="mm")
                    for d_ in range(DT):
                        nc.tensor.matmul(
                            pp[:],
                            lhsT=wall_sb[:, d_, ci * 128 : (ci + 1) * 128],
                            rhs=xt_t[:, d_, :],
                            start=(d_ == 0),
                            stop=(d_ == DT - 1),
                        )
                    raw = stage.tile([128, TB], dt.bfloat16, tag="raw")
                    nc.vector.tensor_copy(raw[:], pp[:])
                    if ci == 3:
                        # v: transpose [c, s] -> natural [s, c] tiles
                        for i in range(TB // 128):
                            tp = mmp.tile([128, 128], dt.bfloat16, tag="mm")
                            nc.tensor.transpose(
                                tp[:], raw[:, i * 128 : (i + 1) * 128], ident_sb[:]
                            )
                            nc.vector.tensor_copy(
                                vn[:, b * TB + i * 128 : b * TB + (i + 1) * 128], tp[:]
                            )
                    else:
                        col, dest, cn, sn = ci, rope_cfg[ci][1], rope_cfg[ci][2], rope_cfg[ci][3]
                        rp = mmp.tile([128, TB], dt.float32, tag="mm")
                        nc.tensor.matmul(rp[:], lhsT=p64_sb[:], rhs=raw[:])
                        a_ = stage.tile([128, TB], dt.bfloat16, tag="ropea")
                        nc.vector.tensor_mul(a_[:], raw[:], rope_sb[cn][:, ts_])
                        b_ = stage.tile([128, TB], dt.bfloat16, tag="ropeb")
                        nc.vector.tensor_mul(b_[:], rp[:], rope_sb[sn][:, ts_])
                        nc.vector.tensor_add(dest[:, ts_], a_[:], b_[:])

            # ---- phase C: attention ----
            for b in range(NB):
                ts_ = slice(b * TB, (b + 1) * TB)
                kts = list(range(max(0, 4 * b - 8), 4 * b + 4))
                n = len(kts)
                for h in range(2):
                    p_t = ptile.tile([128, KT_PER_B * TB], dt.bfloat16, tag="p")
                    for i, kt_ in enumerate(kts):
                        lp = ltp.tile([128, TB], dt.float32, tag="lt")
                        nc.tensor.matmul(
                            lp[:],
                            lhsT=kt_sb[:, kt_ * 128 : (kt_ + 1) * 128],
                            rhs=qt[h][:, ts_],
                        )
                        nc.scalar.activation(
                            out=p_t[:, i * TB : (i + 1) * TB],
                            in_=lp[:],
                            func=mybir.ActivationFunctionType.Tanh,
                            scale=1.0 / SOFT_CAP,
                        )
                    pv = p_t[:, : n * TB]
                    nc.scalar.activation(
                        out=pv, in_=pv,
                        func=mybir.ActivationFunctionType.Exp,
                        scale=SOFT_CAP,
                    )
                    for i, kt_ in enumerate(kts):
                        delta = kt_ - 4 * b
                        if delta <= -5:
                            mi = delta + 8
                        elif delta >= 0:
                            mi = 4 + delta
                        else:
                            continue
                        sl = p_t[:, i * TB : (i + 1) * TB]
                        nc.vector.tensor_mul(sl, sl, masks_sb[:, mi, :])
                    ep = encp.tile([128, TB], dt.float32, tag="enc")
                    for i, kt_ in enumerate(kts):
                        nc.tensor.matmul(
                            ep[:],
                            lhsT=vn[:, kt_ * 128 : (kt_ + 1) * 128],
                            rhs=p_t[:, i * TB : (i + 1) * TB],
                            start=(i == 0),
                            stop=(i == n - 1),
                        )
                    dp = dnp.tile([1, TB], dt.float32, tag="dn")
                    for i in range(n):
                        nc.tensor.matmul(
                            dp[:],
                            lhsT=ones_col[:],
                            rhs=p_t[:, i * TB : (i + 1) * TB],
                            start=(i == 0),
                            stop=(i == n - 1),
                        )
                    rcp = small.tile([1, TB], dt.float32, tag="rcp", bufs=2)
                    rscr = small.tile([1, TB], dt.float32, tag="rscr", bufs=1)
                    nc.vector.reciprocal_approx_accurate(rcp[:], dp[:], rscr[:])
                    rbs = small.tile([128, TB], dt.float32, tag="rbs", bufs=2)
                    nc.gpsimd.partition_broadcast(rbs[:], rcp[:])
                    nc.vector.tensor_mul(enc[h][:, ts_], ep[:], rbs[:])

            # ---- phase D: output projection (2 heads accumulated) ----
            for tt in range(NKT):
                o_sb = outstp.tile([128, D], dt.float32, tag="o")
                for dc in range(D // 512):
                    op = mmp.tile([128, 512], dt.float32, tag="mm")
                    for h in range(2):
                        nc.tensor.matmul(
                            op[:],
                            lhsT=enc[h][:, tt * 128 : (tt + 1) * 128],
                            rhs=ow_sb[:, h, dc * 512 : (dc + 1) * 512],
                            start=(h == 0),
                            stop=(h == 1),
                        )
                    nc.scalar.copy(
                        out=o_sb[:, dc * 512 : (dc + 1) * 512], in_=op[:]
                    )
                nc.sync.dma_start(
                    out=out[tt * 128 : (tt + 1) * 128, :], in_=o_sb[:]
                )

    nc.compile()
    return nc


_NC_CACHE = None


def _get_program():
    global _NC_CACHE
    if _NC_CACHE is None:
        _NC_CACHE = _build_program()
    return _NC_CACHE


def make_in_maps(x, segment_pos, q_w, kv_w, o_w):
    """Host-side shard prep: per-core input dicts."""
    positions = np.asarray(segment_pos)[0].astype(np.float64)
    cos2, sin_s = _rope_tables(positions)
    s = QUERY_PRE_ATTN_SCALAR
    cosq = (cos2 * s).astype(BF16)
    sinq = (sin_s * s).astype(BF16)
    cosk = cos2.astype(BF16)
    sink = sin_s.astype(BF16)

    xt = (
        np.asarray(x)[0].T.astype(BF16).reshape(DT, 128, T).transpose(1, 0, 2)
    )  # [128, DT, T], element [p, d_, t] = x[t, d_*128+p]
    xt = np.ascontiguousarray(xt)

    in_maps = []
    for c in range(N_CORES):
        w_cols = np.concatenate(
            [
                np.asarray(q_w)[2 * c],  # [D, 128]
                np.asarray(q_w)[2 * c + 1],
                np.asarray(kv_w)[0, c],
                np.asarray(kv_w)[1, c],
            ],
            axis=1,
        )  # [D, 512]
        wall = np.ascontiguousarray(
            w_cols.astype(BF16).reshape(DT, 128, 512).transpose(1, 0, 2)
        )  # [128, DT, 512], [p, d_, col] = W[d_*128+p, col]
        ow = np.ascontiguousarray(
            np.stack(
                [np.asarray(o_w)[2 * c], np.asarray(o_w)[2 * c + 1]], axis=1
            ).astype(BF16)
        )  # [128, 2, D]
        in_maps.append(
            {
                "xt": xt,
                "wall": wall,
                "ow": ow,
                "cosq": cosq,
                "sinq": sinq,
                "cosk": cosk,
                "sink": sink,
            }
        )
    return in_maps


def kernel(x, segment_pos, attn_mask, q_w, kv_w, o_w, _collect=None):
    nc = _get_program()
    in_maps = make_in_maps(x, segment_pos, q_w, kv_w, o_w)
    res = run_bass_kernel_spmd(nc, in_maps, core_ids=list(range(N_CORES)))
    if _collect is not None:
        _collect.append(res)
    acc = np.zeros((T, D), dtype=np.float32)
    for r in res.results:
        acc += r["out"]
    return acc.reshape(B, T, D)
